# revision 31
# baseline (speedup 1.0000x reference)
"""Trainium2 Bass kernel for nn_AttributedEncoder (GNN attribute message passing).

Strategy (8 NeuronCores, SPMD, no collectives):
  - Host shards EDGES by destination node: core k owns nodes [k*6272,(k+1)*6272)
    and every edge pointing into that range; host concatenates block outputs.
  - Math: with s_e = exp(lrelu(ent_s[h] + att_s[att])),
       out[n] = elu( (Sum_e s_e*(attW1[att_e] + val[val_e]@W2)) / Sum_e s_e + ent[n] )
    W1 is folded into the att table ON DEVICE (att table is tiny: 2000 rows),
    W2 is applied per 128-node window after aggregation.
  - Edges packed in 128-edge tiles, (window, val-class)-pure.  Per tile one
    SWDGE gather call region pulls BOTH the att row and the val row from a
    combined per-class table (att block replicated in front of each val shard)
    so a single 512-col PE matmul aggregates both halves:
        agg[n, 0:256] += S^T attW1_rows   (+ later (agg val)@W2 accumulated)
        agg[n,256:512] += S^T val_rows
  - S = S0 * s_e where S0 (one-hot hrel masks, bf16) is HOST-precomputed and
    streamed from DRAM; per-edge ent scores come from a per-tile PE matvec
    against the transposed one-hot S0T (also streamed): xe = S0T^T @ enthl.
  - att_s rides the att table rows as bf16 hi/lo columns; the whole score
    pipeline runs batched per super-block: 2 DVE adds + Lrelu + Exp on
    [128, tiles_per_sb], then one tensor_scalar per tile builds S.
  - elu via ACT Relu/Exp (min(x,0) = -relu(-x)) + 2 DVE ops.
"""
import os
import sys
import types

import numpy as np

sys.path.insert(0, "/opt/trn_rl_repo")
if "/root/.axon_site" not in sys.path:
    sys.path.insert(0, "/root/.axon_site")


def _install_trace_hook():
    try:
        import antenv
        if "antenv.axon_hooks" in sys.modules:
            return
        from trn_agent_boot.trn_boot import _ntff_profile_via_ctypes

        hook = _ntff_profile_via_ctypes("/opt/axon/libaxon_pjrt.so")
        mod = types.ModuleType("antenv.axon_hooks")
        mod.get_axon_ntff_profile_hook = lambda: hook
        mod.set_axon_ntff_profile_hook = lambda h: None
        sys.modules["antenv.axon_hooks"] = mod
        antenv.axon_hooks = mod
    except Exception:
        pass


_install_trace_hook()

from concourse import bass, mybir, tile  # noqa: E402
from concourse import bass_utils as _bu  # noqa: E402
from concourse import library_config  # noqa: E402
from concourse.library_overlay import lower_extended_insts  # noqa: E402
from concourse.masks import make_identity  # noqa: E402
from concourse.tile import add_dep_helper  # noqa: E402
from concourse.mybir import AluOpType, ActivationFunctionType  # noqa: E402

_bu.upload_artifacts = lambda tmpdir: f"file://{tmpdir}"

P = 128
D = 256
N_ENT = 50000
N_ATT = 2000
N_VAL = 100000
E_TOT = 300000
N_CORES = 8
NODES_PER_CORE = 6272          # 49 windows of 128; 8*6272 = 50176 >= 50000
NW = NODES_PER_CORE // P       # 49
SBW = 4                        # windows per super-block
VCLS = 4
VSH = 25600                    # val shard size (int16-safe with att block)
NATT_PAD = 2048
CTROWS = NATT_PAD + VSH        # combined table rows per class (27648 < 32768)
CEXT = 384                     # row width (bf16) -> 768B, 256B-aligned
CHUNK_T = 4                    # (legacy; unused)
CHUNK_T2 = 8                   # tiles per split gather call (1024 idxs)
ACT = ActivationFunctionType


def legalize_waits(nc, max_engine_waits=1):
    """Hoist excess sync waits onto standalone EventSemaphore instructions on
    the op's own sequencer engine (queue DMAs encode at most one wait)."""
    wid = 0
    for b in nc.m.functions[0].blocks:
        newinsts = []
        for inst in b.instructions:
            si = getattr(inst, "sync_info", None)
            ow = list(si.on_wait) if si and si.on_wait else []
            qname = getattr(inst, "queue", None)
            is_q = bool(qname) or type(inst).__name__ in (
                "InstDMAGatherAnt", "InstDMAScatterAddAnt", "InstDMACopy", "InstNoOp")
            limit = 1 if is_q else max_engine_waits
            if len(ow) > limit:
                while len(ow) > limit:
                    w, ow = ow[0], ow[1:]
                    es = mybir.InstEventSemaphore(
                        name=f"WAITC-{wid}", engine=inst.engine, ins=[], outs=[])
                    wid += 1
                    es.sync_info = mybir.SyncInfo(on_wait=[w], on_update=[])
                    try:
                        nc.register_instruction(es)
                    except Exception:
                        pass
                    newinsts.append(es)
                si.on_wait = ow
            newinsts.append(inst)
        b.instructions = newinsts


def reassign_swdge_queues(nc, n_queues=4):
    """Tile assigns DMASW sem lanes in final instruction order (mod 8); the
    sim locks each lane to one SWDGE queue. Renumber queue_num in the same
    order so lane L always pairs with queue L % n_queues."""
    cnt = 0
    for b in nc.m.functions[0].blocks:
        for inst in b.instructions:
            if type(inst).__name__ in ("InstDMAGatherAnt", "InstDMAScatterAddAnt"):
                inst.queue_num = cnt % n_queues
                cnt += 1
            elif type(inst).__name__ == "InstDMACopy" and \
                    getattr(inst, "queue", "") == "qPoolDynamic":
                cnt += 1


def _pack16(flat):
    """dma_gather index layout: idxs_ap[p, s] = flat[s*16 + p], first-16-row
    block replicated across the 8 Q7 core groups (128 partitions)."""
    n = len(flat)
    assert n % 16 == 0
    blk = np.asarray(flat, dtype=np.int16).reshape(n // 16, 16).T
    return np.tile(blk, (8, 1))


def _schedule(counts):
    """counts[k][w][c] = edge count. Returns shared tile schedule."""
    T = np.zeros((NW, VCLS), dtype=np.int64)
    for k in range(len(counts)):
        T = np.maximum(T, -(-counts[k] // P))
    for w in range(NW):
        if T[w].sum() == 0:
            T[w][0] = 1
    sbs = [list(range(s, min(s + SBW, NW))) for s in range(0, NW, SBW)]
    tiles = []          # (w, c)
    tile_of = {}        # (w,c) -> [tile ids]
    calls = []          # dict(kind, cls, t0, nt, col0, sb)
    sb_rng = []         # (t_start, t_end) per sb
    icol = 0
    for si, sb in enumerate(sbs):
        sb_t0 = len(tiles)
        for c in range(VCLS):
            run_t0 = len(tiles)
            for w in sb:
                for _ in range(int(T[w][c])):
                    tile_of.setdefault((w, c), []).append(len(tiles))
                    tiles.append((w, c))
            nrun = len(tiles) - run_t0
            t = run_t0
            while nrun > 0:
                nt = min(CHUNK_T, nrun)
                calls.append(dict(kind="cmb", cls=c, t0=t, nt=nt, col0=icol,
                                  sb=si))
                icol += 16 * nt
                t += nt
                nrun -= nt
        sb_rng.append((sb_t0, len(tiles)))
    first_t = {}
    last_t = {}
    for i, (w, c) in enumerate(tiles):
        first_t.setdefault(w, i)
        last_t[w] = i
    max_tsb = max(e - s for s, e in sb_rng)
    return dict(T=T, sbs=sbs, tiles=tiles, tile_of=tile_of, calls=calls,
                n_tiles=len(tiles), icols=icol, sb_rng=sb_rng,
                first_t=first_t, last_t=last_t, max_tsb=max_tsb)


def _host_shard(attribute_triples, ent_feats, sched_only=False):
    """Per-core edge routing."""
    h = np.asarray(attribute_triples[:, 0], dtype=np.int64)
    val = np.asarray(attribute_triples[:, 1], dtype=np.int64)
    att = np.asarray(attribute_triples[:, 2], dtype=np.int64)
    core = h // NODES_PER_CORE
    out = []
    for k in range(N_CORES):
        m = core == k
        hl = h[m] - k * NODES_PER_CORE
        out.append((hl // P, hl % P, val[m] // VSH, val[m] % VSH, att[m]))
    return out


def build_program(sched):
    NT = sched["n_tiles"]
    TSB = sched["max_tsb"]
    nc = bass.Bass(num_swdge_queues=4)
    f32 = mybir.dt.float32
    bf16 = mybir.dt.bfloat16
    ent_d = nc.declare_dram_parameter("ent", [NODES_PER_CORE, D], bf16, isOutput=False)
    attf_d = nc.declare_dram_parameter("attf", [NATT_PAD, D], f32, isOutput=False)
    ctab_d = nc.declare_dram_parameter("ctab", [VCLS * CTROWS, CEXT], bf16, isOutput=False)
    a1r_d = nc.declare_dram_parameter("a1r", [P, D], f32, isOutput=False)
    a2r_d = nc.declare_dram_parameter("a2r", [P, D], f32, isOutput=False)
    br_d = nc.declare_dram_parameter("br", [P, 1], f32, isOutput=False)
    w1_d = nc.declare_dram_parameter("w1", [D, D], bf16, isOutput=False)
    w2_d = nc.declare_dram_parameter("w2", [D, D], bf16, isOutput=False)
    cidx_d = nc.declare_dram_parameter("cidx", [P, sched["icols"]], mybir.dt.int16, isOutput=False)
    fp8 = mybir.dt.float8e4
    s0_d = nc.declare_dram_parameter("s0", [P, NT * P], fp8, isOutput=False)
    s0t_d = nc.declare_dram_parameter("s0t", [P, NT * P], fp8, isOutput=False)
    out_d = nc.declare_dram_parameter("out", [NODES_PER_CORE, D], f32, isOutput=True)

    tiles = sched["tiles"]
    sbs = sched["sbs"]
    calls = sched["calls"]
    sb_rng = sched["sb_rng"]
    first_t, last_t = sched["first_t"], sched["last_t"]
    nsb = len(sbs)
    calls_of_sb = {}
    for ci, call in enumerate(calls):
        calls_of_sb.setdefault(call["sb"], []).append(ci)
    sb_c0 = {}
    sb_cw = {}
    for si2 in range(nsb):
        cls = [calls[ci] for ci in calls_of_sb[si2]]
        sb_c0[si2] = min(c["col0"] for c in cls)
        sb_cw[si2] = sum(16 * c["nt"] for c in cls)
    MAXCW = max(sb_cw.values())
    call_of_tile = {}
    for ci, call in enumerate(calls):
        for b in range(call["nt"]):
            call_of_tile[call["t0"] + b] = (ci, b)

    with tile.TileContext(nc) as tc:
        nc.gpsimd.load_library(library_config.mlp)
        with (
            tc.tile_pool(name="const", bufs=1) as cp,
            tc.tile_pool(name="ph0", bufs=2) as php,
            tc.tile_pool(name="atp", bufs=4) as atp,
            tc.tile_pool(name="gsl", bufs=16) as gsl,
            tc.tile_pool(name="s0p", bufs=2) as s0p,
            tc.tile_pool(name="cxp", bufs=2) as cxp,
            tc.tile_pool(name="s0tp", bufs=2) as s0tp,
            tc.tile_pool(name="ssp", bufs=2) as ssp,
            tc.tile_pool(name="scb", bufs=2) as scb,
            tc.tile_pool(name="entp", bufs=16) as entp,
            tc.tile_pool(name="fin", bufs=2) as fip,
            tc.tile_pool(name="aggp", bufs=3, space="PSUM") as aggp,
            tc.tile_pool(name="tpp", bufs=2, space="PSUM") as tpp,
            tc.tile_pool(name="xrp", bufs=2, space="PSUM") as xrp,
        ):
            # ---- constants ----
            ident = cp.tile([P, P], f32, tag="ident")
            make_identity(nc, ident[:])
            identb = cp.tile([P, P], bf16, tag="identb")
            nc.vector.tensor_copy(out=identb[:], in_=ident[:])
            a1r = cp.tile([P, D], f32, tag="a1r")
            nc.sync.dma_start(out=a1r[:], in_=a1r_d[:])
            a2r = cp.tile([P, D], f32, tag="a2r")
            nc.sync.dma_start(out=a2r[:], in_=a2r_d[:])
            br = cp.tile([P, 1], f32, tag="br")
            nc.sync.dma_start(out=br[:], in_=br_d[:])
            w1s = cp.tile([P, 2, D], bf16, tag="w1s")
            w2s = cp.tile([P, 2, D], bf16, tag="w2s")
            for j in range(2):
                nc.sync.dma_start(out=w1s[:, j, :], in_=w1_d[j * P:(j + 1) * P, :])
                nc.sync.dma_start(out=w2s[:, j, :], in_=w2_d[j * P:(j + 1) * P, :])
            ones_mm = cp.tile([P, 1], bf16, tag="om")
            nc.vector.memset(ones_mm[:], 1.0)
            # cidx is streamed per-SB (see issue_streams)

            nreg = {}
            for call in calls:
                v = 2 * P * call["nt"]
                if v not in nreg:
                    nreg[v] = nc.gpsimd.to_reg(v)

            # ---- phase 0a: att scores (per chunk) ----
            ascol = cp.tile([P, NATT_PAD // P], f32, tag="ascol")
            atiles = []
            for ch in range(NATT_PAD // P):
                atile = atp.tile([P, D], f32, tag="p0a")
                nc.sync.dma_start(out=atile[:], in_=attf_d[ch * P:(ch + 1) * P, :])
                atiles.append(atile)
            for ch in range(NATT_PAD // P):
                atile = atiles[ch]
                scr = php.tile([P, D], f32, tag="p0scr")
                nc.vector.tensor_tensor(out=scr[:], in0=atile[:], in1=a2r[:],
                                        op=AluOpType.mult)
                nc.vector.reduce_sum(out=ascol[:, ch:ch + 1], in_=scr[:],
                                     axis=mybir.AxisListType.X)
                nc.vector.tensor_tensor(out=ascol[:, ch:ch + 1],
                                        in0=ascol[:, ch:ch + 1], in1=br[:],
                                        op=AluOpType.add)
                # transpose att chunk (bf16) and fold in W1
                atbf = php.tile([P, D], bf16, tag="p0ab")
                nc.scalar.activation(out=atbf[:], in_=atile[:], func=ACT.Copy)
                tp0 = tpp.tile([P, D], f32, tag="tp", space="PSUM")
                tms = []
                for j in range(2):
                    tm = nc.tensor.matmul(out=tp0[:, j * P:(j + 1) * P],
                                          lhsT=atbf[:, j * P:(j + 1) * P],
                                          rhs=identb[:],
                                          start=(j == 0), stop=(j == 1),
                                          skip_group_check=True)
                    tms.append(tm)
                add_dep_helper(tms[1].ins, tms[0].ins, sync=False,
                               reason="psum bank-clear order")
                atT = php.tile([P, D], bf16, tag="p0at")
                nc.scalar.activation(out=atT[:], in_=tp0[:], func=ACT.Copy)
                aw = aggp.tile([P, 4 * P], f32, tag="agg", space="PSUM")
                for j in range(2):
                    nc.tensor.matmul(out=aw[:, 0:D], lhsT=atT[:, j * P:(j + 1) * P],
                                     rhs=w1s[:, j, :], start=(j == 0), stop=(j == 1))
                abf = php.tile([P, D + 2], bf16, tag="p0w")
                nc.scalar.activation(out=abf[:, 0:D], in_=aw[:, 0:D], func=ACT.Copy)
                # riders: att_s hi/lo
                shi = php.tile([P, 1], bf16, tag="p0h")
                nc.vector.tensor_copy(out=shi[:], in_=ascol[:, ch:ch + 1])
                shif = php.tile([P, 1], f32, tag="p0hf")
                nc.vector.tensor_copy(out=shif[:], in_=shi[:])
                slo = php.tile([P, 1], f32, tag="p0l")
                nc.vector.tensor_tensor(out=slo[:], in0=ascol[:, ch:ch + 1],
                                        in1=shif[:], op=AluOpType.subtract)
                nc.vector.tensor_copy(out=abf[:, D:D + 1], in_=shi[:])
                nc.vector.tensor_copy(out=abf[:, D + 1:D + 2], in_=slo[:])
                nc.scalar.dma_start(
                    out=ctab_d[ch * P:(ch + 1) * P, 0:D + 2], in_=abf[:])

            for c in range(1, VCLS):
                nc.sync.dma_start(
                    out=ctab_d[c * CTROWS:c * CTROWS + NATT_PAD, 0:D + 2],
                    in_=ctab_d[0:NATT_PAD, 0:D + 2])

            # ---- ent scores are computed in-loop (prep_ent_*) ----
            entsc = cp.tile([P, NW], f32, tag="entsc")
            enthl = cp.tile([P, NW, 2], bf16, tag="enthl")
            ehif = cp.tile([P, NW], f32, tag="ehif")
            elo = cp.tile([P, NW], f32, tag="elo")
            ent_tiles = {}

            def prep_ent_dma(si):
                for w in sbs[si]:
                    et = entp.tile([P, D], bf16, tag="entw")
                    nc.sync.dma_start(out=et[:], in_=ent_d[w * P:(w + 1) * P, :])
                    ent_tiles[w] = et

            def prep_ent_compute(si):
                w0, w1 = sbs[si][0], sbs[si][-1] + 1
                for w in sbs[si]:
                    scr2 = php.tile([P, D], f32, tag="p0s2")
                    nc.vector.tensor_tensor(out=scr2[:], in0=ent_tiles[w][:],
                                            in1=a1r[:], op=AluOpType.mult)
                    nc.vector.reduce_sum(out=entsc[:, w:w + 1], in_=scr2[:],
                                         axis=mybir.AxisListType.X)
                nc.vector.tensor_copy(out=enthl[:, w0:w1, 0], in_=entsc[:, w0:w1])
                nc.vector.tensor_copy(out=ehif[:, w0:w1], in_=enthl[:, w0:w1, 0])
                nc.vector.tensor_tensor(out=elo[:, w0:w1], in0=entsc[:, w0:w1],
                                        in1=ehif[:, w0:w1], op=AluOpType.subtract)
                nc.vector.tensor_copy(out=enthl[:, w0:w1, 1], in_=elo[:, w0:w1])

            # ---- helpers for main loop state ----
            slot_of_call = {}
            slot_sb_issued = set([0])
            cx_of_sb = {}
            s0_of_sb = {}
            s0t_of_sb = {}
            ssb_of_sb = {}
            xr_of_sb = {}
            svt_of_sb = {}
            qn = [0]

            def issue_gathers(si):
                for ci in calls_of_sb[si]:
                    call = calls[ci]
                    nt = call["nt"]
                    cc0 = call["col0"] - sb_c0[si]
                    c = call["cls"]
                    slot = gsl.tile([P, 2 * nt, CEXT], bf16, tag="gsl",
                                    padded_shape=[P, 2 * CHUNK_T, CEXT])
                    nc.gpsimd.dma_gather(
                        out_ap=slot[:], in_ap=ctab_d[c * CTROWS:(c + 1) * CTROWS, :],
                        idxs_ap=cx_of_sb[si][:, cc0:cc0 + 16 * nt],
                        num_idxs=2 * P * nt, num_idxs_reg=nreg[2 * P * nt],
                        elem_size=CEXT, single_packet=False,
                        queue_num=qn[0] % 4)
                    qn[0] += 1
                    slot_of_call[ci] = slot

            def issue_streams(si):
                t0, t1 = sb_rng[si]
                n = (t1 - t0) * P
                s0s = s0p.tile([P, TSB * P], fp8, tag="s0s")
                nc.sync.dma_start(out=s0s[:, 0:n], in_=s0_d[:, t0 * P:t1 * P])
                s0ts = s0tp.tile([P, TSB * P], fp8, tag="s0ts")
                nc.sync.dma_start(out=s0ts[:, 0:n], in_=s0t_d[:, t0 * P:t1 * P])
                cxs = cxp.tile([P, MAXCW], mybir.dt.int16, tag="cxs")
                nc.sync.dma_start(out=cxs[:, 0:sb_cw[si]],
                                  in_=cidx_d[:, sb_c0[si]:sb_c0[si] + sb_cw[si]])
                cx_of_sb[si] = cxs
                s0_of_sb[si] = s0s
                s0t_of_sb[si] = s0ts

            def issue_xe(si):
                """Per-tile PE matvec: xr[:, ti, 0:2] = S0T_t^T @ enthl[:, w, :].
                Also reserves rowsum columns [:, TSB + wi, 0]."""
                t0, t1 = sb_rng[si]
                xr = xrp.tile([P, 2 * TSB + SBW], f32, tag="xr", space="PSUM")
                xr_of_sb[si] = xr
                s0ts = s0t_of_sb[si]
                first = None
                for t in range(t0, t1):
                    ti = t - t0
                    w = tiles[t][0]
                    wloc = w - sbs[si][0]
                    mm = nc.tensor.matmul(
                        out=xr[:, 2 * ti:2 * ti + 2],
                        lhsT=s0ts[:, ti * P:(ti + 1) * P],
                        rhs=enthl[:, w, :],
                        start=(first is None), stop=(t == t1 - 1),
                        skip_group_check=True)
                    if first is None:
                        first = mm
                    else:
                        add_dep_helper(mm.ins, first.ins, sync=False,
                                       reason="xr bank-clear order")
                return first

            xr_first_of_sb = {}

            def issue_score(si):
                """Batched score pipeline for SB si -> svt (bf16 [P, T])."""
                t0, t1 = sb_rng[si]
                T = t1 - t0
                xr = xr_of_sb[si]
                xa = scb.tile([P, TSB], f32, tag="xa")
                for ci in calls_of_sb[si]:
                    call = calls[ci]
                    slot = slot_of_call[ci]
                    r0 = call["t0"] - t0
                    nt = call["nt"]
                    sl4 = slot[:].rearrange("p (t two) e -> p t two e", two=2)
                    nc.vector.tensor_tensor(
                        out=xa[:, r0:r0 + nt],
                        in0=sl4[:, :, 0, D], in1=sl4[:, :, 0, D + 1],
                        op=AluOpType.add)
                xs1 = scb.tile([P, TSB], f32, tag="xs1")
                xrv = xr[:, 0:2 * T].rearrange("p (t two) -> p t two", two=2)
                nc.vector.tensor_tensor(out=xs1[:, 0:T], in0=xa[:, 0:T],
                                        in1=xrv[:, :, 0], op=AluOpType.add)
                xs2 = scb.tile([P, TSB], f32, tag="xs2")
                nc.vector.tensor_tensor(out=xs2[:, 0:T], in0=xs1[:, 0:T],
                                        in1=xrv[:, :, 1], op=AluOpType.add)
                x2 = scb.tile([P, TSB], f32, tag="x2")
                nc.vector.tensor_scalar(out=x2[:, 0:T], in0=xs2[:, 0:T],
                                        scalar1=0.2, scalar2=None,
                                        op0=AluOpType.mult)
                lr = scb.tile([P, TSB], f32, tag="lr")
                nc.vector.tensor_tensor(out=lr[:, 0:T], in0=xs2[:, 0:T],
                                        in1=x2[:, 0:T], op=AluOpType.max)
                svt = scb.tile([P, TSB], f32, tag="svt")
                nc.scalar.activation(out=svt[:, 0:T], in_=lr[:, 0:T],
                                     func=ACT.Exp)
                svt_of_sb[si] = svt

            def issue_sscale(si):
                t0, t1 = sb_rng[si]
                svt = svt_of_sb[si]
                s0s = s0_of_sb[si]
                ssb = ssp.tile([P, TSB * P], bf16, tag="ssb")
                for t in range(t0, t1):
                    ti = t - t0
                    nc.vector.tensor_scalar(
                        out=ssb[:, ti * P:(ti + 1) * P],
                        in0=s0s[:, ti * P:(ti + 1) * P],
                        scalar1=svt[:, ti:ti + 1], scalar2=None,
                        op0=AluOpType.mult)
                ssb_of_sb[si] = ssb

            def finalize_window(si, w, wloc, agg, agg_first, ent_tile, rr_stop):
                xr = xr_of_sb[si]
                # 1/rowsum
                rr = fip.tile([P, 1], f32, tag="rr")
                nc.vector.tensor_scalar(out=rr[:], in0=xr[:, 2 * TSB + wloc:2 * TSB + wloc + 1],
                                        scalar1=1e-30, scalar2=None,
                                        op0=AluOpType.max)
                nc.vector.reciprocal(out=rr[:], in_=rr[:])
                # val half -> transpose -> @W2 accumulated into att half
                aggs2 = fip.tile([P, D], bf16, tag="aggs2")
                nc.scalar.activation(out=aggs2[:], in_=agg[:, D:2 * D], func=ACT.Copy)
                tp = tpp.tile([P, D], f32, tag="tp", space="PSUM")
                tms = []
                for j in range(2):
                    tm = nc.tensor.matmul(out=tp[:, j * P:(j + 1) * P],
                                          lhsT=aggs2[:, j * P:(j + 1) * P],
                                          rhs=identb[:], start=(j == 0),
                                          stop=(j == 1), skip_group_check=True)
                    tms.append(tm)
                add_dep_helper(tms[1].ins, tms[0].ins, sync=False,
                               reason="psum bank-clear order")
                tps = fip.tile([P, D], bf16, tag="tps")
                nc.scalar.activation(out=tps[:], in_=tp[:], func=ACT.Copy)
                for j in range(2):
                    mm = nc.tensor.matmul(out=agg[:, 0:D],
                                          lhsT=tps[:, j * P:(j + 1) * P],
                                          rhs=w2s[:, j, :], start=False,
                                          stop=(j == 1), skip_group_check=True)
                    add_dep_helper(mm.ins, agg_first.ins, sync=False,
                                   reason="agg bank-clear order")
                # fin = agg*rr + ent ; out = elu(fin)
                fin1 = fip.tile([P, D], f32, tag="fin1")
                nc.scalar.activation(out=fin1[:], in_=agg[:, 0:D], func=ACT.Copy,
                                     scale=rr[:])
                fin2 = fip.tile([P, D], f32, tag="fin2")
                nc.vector.tensor_tensor(out=fin2[:], in0=fin1[:], in1=ent_tile[:],
                                        op=AluOpType.add)
                rng = fip.tile([P, D], f32, tag="rng")
                nc.scalar.activation(out=rng[:], in_=fin2[:], func=ACT.Relu,
                                     scale=-1.0)
                ex = fip.tile([P, D], f32, tag="ex")
                nc.scalar.activation(out=ex[:], in_=rng[:], func=ACT.Exp,
                                     scale=-1.0)
                rl = fip.tile([P, D], f32, tag="rl")
                nc.scalar.activation(out=rl[:], in_=fin2[:], func=ACT.Relu)
                fo = fip.tile([P, D], f32, tag="fo")
                nc.vector.tensor_tensor(out=fo[:], in0=rl[:], in1=ex[:],
                                        op=AluOpType.add)
                fo2 = fip.tile([P, D], f32, tag="fo2")
                nc.scalar.activation(out=fo2[:], in_=fo[:], func=ACT.Copy,
                                     bias=-1.0)
                nc.scalar.dma_start(out=out_d[w * P:(w + 1) * P, :], in_=fo2[:])

            # ---- prologue ----
            max_sbs = int(os.environ.get("K_MAX_SBS", "9999"))
            pro = int(os.environ.get("K_PROLOGUE", "7"))
            for s in range(min(3, nsb)):
                prep_ent_dma(s)
            for s in range(min(3, nsb)):
                prep_ent_compute(s)
            for s in range(min(2, nsb)):
                issue_streams(s)
            issue_gathers(0)
            xr_first_of_sb[0] = issue_xe(0)

            # ---- main loop ----
            for si, sb in enumerate(sbs):
                if si >= max_sbs:
                    break
                t0, t1 = sb_rng[si]
                if si + 1 < nsb and si + 1 not in s0_of_sb:
                    issue_streams(si + 1)
                if si + 3 < nsb:
                    prep_ent_dma(si + 3)
                issue_score(si)
                issue_sscale(si)
                if si + 3 < nsb:
                    prep_ent_compute(si + 3)
                ssb = ssb_of_sb[si]
                xr = xr_of_sb[si]
                pending = []
                agg_of_w = {}
                aggfirst_of_w = {}
                for wi, w in enumerate(sb):
                    agg = aggp.tile([P, 4 * P], f32, tag="agg", space="PSUM")
                    agg_of_w[w] = agg
                    wtiles = [t for t in range(t0, t1) if tiles[t][0] == w]
                    firstmm = None
                    for t in wtiles:
                        ti = t - t0
                        ci, b = call_of_tile[t]
                        slot = slot_of_call[ci]
                        mm = nc.tensor.matmul(out=agg[:, 0:4 * P],
                                              lhsT=ssb[:, ti * P:(ti + 1) * P],
                                              rhs=slot[:, 2 * b:2 * b + 2, 0:D],
                                              start=(firstmm is None),
                                              stop=False, skip_group_check=True)
                        if firstmm is None:
                            firstmm = mm
                        else:
                            add_dep_helper(mm.ins, firstmm.ins, sync=False,
                                           reason="agg bank-clear order")
                        is_last_rs = (wi == len(sb) - 1 and t == wtiles[-1])
                        rs = nc.tensor.matmul(out=xr[:, 2 * TSB + wi:2 * TSB + wi + 1],
                                              lhsT=ssb[:, ti * P:(ti + 1) * P],
                                              rhs=ones_mm[:], start=False,
                                              stop=is_last_rs,
                                              skip_group_check=True)
                        add_dep_helper(rs.ins, xr_first_of_sb[si].ins, sync=False,
                                       reason="xr bank-clear order")
                    aggfirst_of_w[w] = firstmm
                    if si + 1 < nsb and wi == 0 and si + 1 not in slot_sb_issued:
                        issue_gathers(si + 1)
                        slot_sb_issued.add(si + 1)
                    if si + 1 < nsb and wi == 0 and si + 1 not in xr_first_of_sb:
                        xr_first_of_sb[si + 1] = issue_xe(si + 1)
                    if pending:
                        pw, pwloc = pending.pop()
                        finalize_window(si, pw, pwloc, agg_of_w[pw],
                                        aggfirst_of_w[pw], ent_tiles[pw], None)
                    pending.append((w, wi))
                if pending:
                    pw, pwloc = pending.pop()
                    finalize_window(si, pw, pwloc, agg_of_w[pw],
                                    aggfirst_of_w[pw], ent_tiles[pw], None)

    lower_extended_insts(nc)
    reassign_swdge_queues(nc)
    legalize_waits(nc)
    return nc
def _per_core_inputs(sched, shards, att_feats, val_feats, ent_feats, a_w, a_b, W):
    import ml_dtypes
    NT = sched["n_tiles"]
    attf = np.zeros((NATT_PAD, D), dtype=np.float32)
    attf[:N_ATT] = np.asarray(att_feats, dtype=np.float32)
    ctab = np.zeros((VCLS * CTROWS, CEXT), dtype=ml_dtypes.bfloat16)
    valbf = np.asarray(val_feats, dtype=np.float32).astype(ml_dtypes.bfloat16)
    for c in range(VCLS):
        nrow = min(VSH, N_VAL - c * VSH)
        ctab[c * CTROWS + NATT_PAD:c * CTROWS + NATT_PAD + nrow, 0:D] = \
            valbf[c * VSH:c * VSH + nrow]
    entp_full = np.zeros((N_CORES * NODES_PER_CORE, D), dtype=ml_dtypes.bfloat16)
    entp_full[:N_ENT] = np.asarray(ent_feats,
                                   dtype=np.float32).astype(ml_dtypes.bfloat16)
    a_w = np.asarray(a_w, dtype=np.float32)
    a1r = np.tile(a_w[0, :D][None, :], (P, 1)).astype(np.float32)
    a2r = np.tile(a_w[0, D:][None, :], (P, 1)).astype(np.float32)
    br = np.full((P, 1), float(np.asarray(a_b).reshape(-1)[0]), dtype=np.float32)
    Wf = np.asarray(W, dtype=np.float32)
    w1 = Wf[:D].astype(ml_dtypes.bfloat16)
    w2 = Wf[D:].astype(ml_dtypes.bfloat16)

    in_maps = []
    for k in range(N_CORES):
        w_arr, hrel_arr, cls_arr, vloc_arr, att_arr = shards[k]
        hrelf = np.full((NT, P), -1, dtype=np.int64)   # -1 = pad
        vli = np.zeros((NT, P), dtype=np.int64)
        ati = np.zeros((NT, P), dtype=np.int64)
        order = np.lexsort((cls_arr, w_arr))
        wc_sorted = list(zip(w_arr[order], cls_arr[order]))
        i = 0
        nE = len(order)
        while i < nE:
            w0, c0 = wc_sorted[i]
            j = i
            while j < nE and wc_sorted[j] == (w0, c0):
                j += 1
            idxs = order[i:j]
            tlist = sched["tile_of"][(w0, c0)]
            assert len(idxs) <= len(tlist) * P, (k, w0, c0, len(idxs))
            for q, e in enumerate(idxs):
                t = tlist[q // P]
                p = q % P
                hrelf[t, p] = hrel_arr[e]
                vli[t, p] = vloc_arr[e]
                ati[t, p] = att_arr[e]
            i = j
        cidx = np.zeros((P, sched["icols"]), dtype=np.int16)
        for call in sched["calls"]:
            t0, nt, c0 = call["t0"], call["nt"], call["col0"]
            flat = np.stack([ati[t0:t0 + nt],
                             NATT_PAD + vli[t0:t0 + nt]], axis=1).reshape(-1)
            cidx[:, c0:c0 + 16 * nt] = _pack16(flat)
        # one-hot streams
        s0 = np.zeros((P, NT * P), dtype=ml_dtypes.float8_e4m3)
        s0t = np.zeros((P, NT * P), dtype=ml_dtypes.float8_e4m3)
        tt, pp = np.nonzero(hrelf >= 0)
        hh = hrelf[tt, pp]
        s0[pp, tt * P + hh] = 1.0
        s0t[hh, tt * P + pp] = 1.0
        in_maps.append(dict(
            ent=entp_full[k * NODES_PER_CORE:(k + 1) * NODES_PER_CORE],
            attf=attf, ctab=ctab, a1r=a1r, a2r=a2r, br=br,
            w1=w1, w2=w2, cidx=cidx, s0=s0, s0t=s0t,
        ))
    return in_maps


def kernel(attribute_triples, att_feats, val_feats, ent_feats, a_w, a_b, W):
    shards = _host_shard(attribute_triples, ent_feats)
    counts = []
    for k in range(N_CORES):
        w_arr, hrel_arr, cls_arr, vloc_arr, att_arr = shards[k]
        cnt = np.zeros((NW, VCLS), dtype=np.int64)
        np.add.at(cnt, (w_arr, cls_arr), 1)
        counts.append(cnt)
    sched = _schedule(counts)

    nc = build_program(sched)
    in_maps = _per_core_inputs(sched, shards, att_feats, val_feats, ent_feats,
                               a_w, a_b, W)
    trace = os.environ.get("KERNEL_TRACE", "0") == "1"
    res = _bu.run_bass_kernel_spmd(nc, in_maps, list(range(N_CORES)), trace=trace)
    if trace and res.exec_time_ns:
        print(f"HW exec time: {res.exec_time_ns} ns")
    out = np.concatenate([res.results[k]["out"] for k in range(N_CORES)], axis=0)
    return np.ascontiguousarray(out[:N_ENT]).astype(np.float32)


# revision 32
# speedup vs baseline: 1.0604x; 1.0604x over previous
"""Trainium2 Bass kernel for nn_AttributedEncoder (GNN attribute message passing).

Strategy (8 NeuronCores, SPMD, no collectives):
  - Host shards EDGES by destination node: core k owns nodes [k*6272,(k+1)*6272)
    and every edge pointing into that range; host concatenates block outputs.
  - Math: with s_e = exp(lrelu(ent_s[h] + att_s[att])),
       out[n] = elu( (Sum_e s_e*(attW1[att_e] + val[val_e]@W2)) / Sum_e s_e + ent[n] )
    W1 is folded into the att table ON DEVICE (att table is tiny: 2000 rows),
    W2 is applied per 128-node window after aggregation.
  - Edges packed in 128-edge tiles, (window, val-class)-pure.  Per tile one
    SWDGE gather call region pulls BOTH the att row and the val row from a
    combined per-class table (att block replicated in front of each val shard)
    so a single 512-col PE matmul aggregates both halves:
        agg[n, 0:256] += S^T attW1_rows   (+ later (agg val)@W2 accumulated)
        agg[n,256:512] += S^T val_rows
  - S = S0 * s_e where S0 (one-hot hrel masks, bf16) is HOST-precomputed and
    streamed from DRAM; per-edge ent scores come from a per-tile PE matvec
    against the transposed one-hot S0T (also streamed): xe = S0T^T @ enthl.
  - att_s rides the att table rows as bf16 hi/lo columns; the whole score
    pipeline runs batched per super-block: 2 DVE adds + Lrelu + Exp on
    [128, tiles_per_sb], then one tensor_scalar per tile builds S.
  - elu via ACT Relu/Exp (min(x,0) = -relu(-x)) + 2 DVE ops.
"""
import os
import sys
import types

import numpy as np

sys.path.insert(0, "/opt/trn_rl_repo")
if "/root/.axon_site" not in sys.path:
    sys.path.insert(0, "/root/.axon_site")


def _install_trace_hook():
    try:
        import antenv
        if "antenv.axon_hooks" in sys.modules:
            return
        from trn_agent_boot.trn_boot import _ntff_profile_via_ctypes

        hook = _ntff_profile_via_ctypes("/opt/axon/libaxon_pjrt.so")
        mod = types.ModuleType("antenv.axon_hooks")
        mod.get_axon_ntff_profile_hook = lambda: hook
        mod.set_axon_ntff_profile_hook = lambda h: None
        sys.modules["antenv.axon_hooks"] = mod
        antenv.axon_hooks = mod
    except Exception:
        pass


_install_trace_hook()

from concourse import bass, mybir, tile  # noqa: E402
from concourse import bass_utils as _bu  # noqa: E402
from concourse import library_config  # noqa: E402
from concourse.library_overlay import lower_extended_insts  # noqa: E402
from concourse.masks import make_identity  # noqa: E402
from concourse.tile import add_dep_helper  # noqa: E402
from concourse.mybir import AluOpType, ActivationFunctionType  # noqa: E402

_bu.upload_artifacts = lambda tmpdir: f"file://{tmpdir}"

P = 128
D = 256
N_ENT = 50000
N_ATT = 2000
N_VAL = 100000
E_TOT = 300000
N_CORES = 8
NODES_PER_CORE = 6272          # 49 windows of 128; 8*6272 = 50176 >= 50000
NW = NODES_PER_CORE // P       # 49
SBW = 4                        # windows per super-block
VCLS = 4
VSH = 25600                    # val shard size (int16-safe with att block)
NATT_PAD = 2048
CTROWS = NATT_PAD + VSH        # combined table rows per class (27648 < 32768)
CEXT = 384                     # row width (bf16) -> 768B, 256B-aligned
CHUNK_T = 4                    # (legacy; unused)
CHUNK_T2 = 8                   # tiles per split gather call (1024 idxs)
ACT = ActivationFunctionType


def legalize_waits(nc, max_engine_waits=1):
    """Hoist excess sync waits onto standalone EventSemaphore instructions on
    the op's own sequencer engine (queue DMAs encode at most one wait)."""
    wid = 0
    for b in nc.m.functions[0].blocks:
        newinsts = []
        for inst in b.instructions:
            si = getattr(inst, "sync_info", None)
            ow = list(si.on_wait) if si and si.on_wait else []
            qname = getattr(inst, "queue", None)
            is_q = bool(qname) or type(inst).__name__ in (
                "InstDMAGatherAnt", "InstDMAScatterAddAnt", "InstDMACopy", "InstNoOp")
            limit = 1 if is_q else max_engine_waits
            if len(ow) > limit:
                while len(ow) > limit:
                    w, ow = ow[0], ow[1:]
                    es = mybir.InstEventSemaphore(
                        name=f"WAITC-{wid}", engine=inst.engine, ins=[], outs=[])
                    wid += 1
                    es.sync_info = mybir.SyncInfo(on_wait=[w], on_update=[])
                    try:
                        nc.register_instruction(es)
                    except Exception:
                        pass
                    newinsts.append(es)
                si.on_wait = ow
            newinsts.append(inst)
        b.instructions = newinsts


def reassign_swdge_queues(nc, n_queues=4):
    """Tile assigns DMASW sem lanes in final instruction order (mod 8); the
    sim locks each lane to one SWDGE queue. Renumber queue_num in the same
    order so lane L always pairs with queue L % n_queues."""
    cnt = 0
    for b in nc.m.functions[0].blocks:
        for inst in b.instructions:
            if type(inst).__name__ in ("InstDMAGatherAnt", "InstDMAScatterAddAnt"):
                inst.queue_num = cnt % n_queues
                cnt += 1
            elif type(inst).__name__ == "InstDMACopy" and \
                    getattr(inst, "queue", "") == "qPoolDynamic":
                cnt += 1


def _pack16(flat):
    """dma_gather index layout: idxs_ap[p, s] = flat[s*16 + p], first-16-row
    block replicated across the 8 Q7 core groups (128 partitions)."""
    n = len(flat)
    assert n % 16 == 0
    blk = np.asarray(flat, dtype=np.int16).reshape(n // 16, 16).T
    return np.tile(blk, (8, 1))


def _schedule(counts):
    """counts[k][w][c] = edge count. Returns shared tile schedule."""
    T = np.zeros((NW, VCLS), dtype=np.int64)
    for k in range(len(counts)):
        T = np.maximum(T, -(-counts[k] // P))
    for w in range(NW):
        if T[w].sum() == 0:
            T[w][0] = 1
    sbs = [list(range(s, min(s + SBW, NW))) for s in range(0, NW, SBW)]
    tiles = []          # (w, c)
    tile_of = {}        # (w,c) -> [tile ids]
    calls = []          # dict(kind, cls, t0, nt, col0, sb)
    sb_rng = []         # (t_start, t_end) per sb
    icol = 0
    for si, sb in enumerate(sbs):
        sb_t0 = len(tiles)
        for c in range(VCLS):
            run_t0 = len(tiles)
            for w in sb:
                for _ in range(int(T[w][c])):
                    tile_of.setdefault((w, c), []).append(len(tiles))
                    tiles.append((w, c))
            nrun = len(tiles) - run_t0
            t = run_t0
            while nrun > 0:
                nt = min(CHUNK_T, nrun)
                calls.append(dict(kind="cmb", cls=c, t0=t, nt=nt, col0=icol,
                                  sb=si))
                icol += 16 * nt
                t += nt
                nrun -= nt
        sb_rng.append((sb_t0, len(tiles)))
    first_t = {}
    last_t = {}
    for i, (w, c) in enumerate(tiles):
        first_t.setdefault(w, i)
        last_t[w] = i
    max_tsb = max(e - s for s, e in sb_rng)
    return dict(T=T, sbs=sbs, tiles=tiles, tile_of=tile_of, calls=calls,
                n_tiles=len(tiles), icols=icol, sb_rng=sb_rng,
                first_t=first_t, last_t=last_t, max_tsb=max_tsb)


def _host_shard(attribute_triples, ent_feats, sched_only=False):
    """Per-core edge routing."""
    h = np.asarray(attribute_triples[:, 0], dtype=np.int64)
    val = np.asarray(attribute_triples[:, 1], dtype=np.int64)
    att = np.asarray(attribute_triples[:, 2], dtype=np.int64)
    core = h // NODES_PER_CORE
    out = []
    for k in range(N_CORES):
        m = core == k
        hl = h[m] - k * NODES_PER_CORE
        out.append((hl // P, hl % P, val[m] // VSH, val[m] % VSH, att[m]))
    return out


def build_program(sched):
    NT = sched["n_tiles"]
    TSB = sched["max_tsb"]
    nc = bass.Bass(num_swdge_queues=4)
    f32 = mybir.dt.float32
    bf16 = mybir.dt.bfloat16
    ent_d = nc.declare_dram_parameter("ent", [NODES_PER_CORE, D], bf16, isOutput=False)
    attf_d = nc.declare_dram_parameter("attf", [NATT_PAD, D], f32, isOutput=False)
    ctab_d = nc.declare_dram_parameter("ctab", [VCLS * CTROWS, CEXT], bf16, isOutput=False)
    a1r_d = nc.declare_dram_parameter("a1r", [P, D], f32, isOutput=False)
    a2r_d = nc.declare_dram_parameter("a2r", [P, D], f32, isOutput=False)
    br_d = nc.declare_dram_parameter("br", [P, 1], f32, isOutput=False)
    w1_d = nc.declare_dram_parameter("w1", [D, D], bf16, isOutput=False)
    w2_d = nc.declare_dram_parameter("w2", [D, D], bf16, isOutput=False)
    cidx_d = nc.declare_dram_parameter("cidx", [P, sched["icols"]], mybir.dt.int16, isOutput=False)
    fp8 = mybir.dt.float8e4
    s0_d = nc.declare_dram_parameter("s0", [P, NT * P], fp8, isOutput=False)
    s0t_d = nc.declare_dram_parameter("s0t", [P, NT * P], bf16, isOutput=False)
    out_d = nc.declare_dram_parameter("out", [NODES_PER_CORE, D], f32, isOutput=True)

    tiles = sched["tiles"]
    sbs = sched["sbs"]
    calls = sched["calls"]
    sb_rng = sched["sb_rng"]
    first_t, last_t = sched["first_t"], sched["last_t"]
    nsb = len(sbs)
    calls_of_sb = {}
    for ci, call in enumerate(calls):
        calls_of_sb.setdefault(call["sb"], []).append(ci)
    sb_c0 = {}
    sb_cw = {}
    for si2 in range(nsb):
        cls = [calls[ci] for ci in calls_of_sb[si2]]
        sb_c0[si2] = min(c["col0"] for c in cls)
        sb_cw[si2] = sum(16 * c["nt"] for c in cls)
    MAXCW = max(sb_cw.values())
    call_of_tile = {}
    for ci, call in enumerate(calls):
        for b in range(call["nt"]):
            call_of_tile[call["t0"] + b] = (ci, b)

    with tile.TileContext(nc) as tc:
        nc.gpsimd.load_library(library_config.mlp)
        with (
            tc.tile_pool(name="const", bufs=1) as cp,
            tc.tile_pool(name="ph0", bufs=2) as php,
            tc.tile_pool(name="atp", bufs=4) as atp,
            tc.tile_pool(name="gsl", bufs=16) as gsl,
            tc.tile_pool(name="s0p", bufs=2) as s0p,
            tc.tile_pool(name="cxp", bufs=2) as cxp,
            tc.tile_pool(name="s0tp", bufs=2) as s0tp,
            tc.tile_pool(name="ssp", bufs=2) as ssp,
            tc.tile_pool(name="scb", bufs=2) as scb,
            tc.tile_pool(name="entp", bufs=16) as entp,
            tc.tile_pool(name="fin", bufs=2) as fip,
            tc.tile_pool(name="aggp", bufs=3, space="PSUM") as aggp,
            tc.tile_pool(name="tpp", bufs=2, space="PSUM") as tpp,
            tc.tile_pool(name="xrp", bufs=2, space="PSUM") as xrp,
        ):
            # ---- constants ----
            ident = cp.tile([P, P], f32, tag="ident")
            make_identity(nc, ident[:])
            identb = cp.tile([P, P], bf16, tag="identb")
            nc.vector.tensor_copy(out=identb[:], in_=ident[:])
            a1r = cp.tile([P, D], f32, tag="a1r")
            nc.sync.dma_start(out=a1r[:], in_=a1r_d[:])
            a2r = cp.tile([P, D], f32, tag="a2r")
            nc.sync.dma_start(out=a2r[:], in_=a2r_d[:])
            br = cp.tile([P, 1], f32, tag="br")
            nc.sync.dma_start(out=br[:], in_=br_d[:])
            w1s = cp.tile([P, 2, D], bf16, tag="w1s")
            w2s = cp.tile([P, 2, D], bf16, tag="w2s")
            for j in range(2):
                nc.sync.dma_start(out=w1s[:, j, :], in_=w1_d[j * P:(j + 1) * P, :])
                nc.sync.dma_start(out=w2s[:, j, :], in_=w2_d[j * P:(j + 1) * P, :])
            ones_mm = cp.tile([P, 1], bf16, tag="om")
            nc.vector.memset(ones_mm[:], 1.0)
            # cidx is streamed per-SB (see issue_streams)

            nreg = {}
            for call in calls:
                v = 2 * P * call["nt"]
                if v not in nreg:
                    nreg[v] = nc.gpsimd.to_reg(v)

            # ---- phase 0a: att scores (per chunk) ----
            ascol = cp.tile([P, NATT_PAD // P], f32, tag="ascol")
            atiles = []
            for ch in range(NATT_PAD // P):
                atile = atp.tile([P, D], f32, tag="p0a")
                nc.sync.dma_start(out=atile[:], in_=attf_d[ch * P:(ch + 1) * P, :])
                atiles.append(atile)
            for ch in range(NATT_PAD // P):
                atile = atiles[ch]
                scr = php.tile([P, D], f32, tag="p0scr")
                nc.vector.tensor_tensor(out=scr[:], in0=atile[:], in1=a2r[:],
                                        op=AluOpType.mult)
                nc.vector.reduce_sum(out=ascol[:, ch:ch + 1], in_=scr[:],
                                     axis=mybir.AxisListType.X)
                nc.vector.tensor_tensor(out=ascol[:, ch:ch + 1],
                                        in0=ascol[:, ch:ch + 1], in1=br[:],
                                        op=AluOpType.add)
                # transpose att chunk (bf16) and fold in W1
                atbf = php.tile([P, D], bf16, tag="p0ab")
                nc.scalar.activation(out=atbf[:], in_=atile[:], func=ACT.Copy)
                tp0 = tpp.tile([P, D], f32, tag="tp", space="PSUM")
                tms = []
                for j in range(2):
                    tm = nc.tensor.matmul(out=tp0[:, j * P:(j + 1) * P],
                                          lhsT=atbf[:, j * P:(j + 1) * P],
                                          rhs=identb[:],
                                          start=(j == 0), stop=(j == 1),
                                          skip_group_check=True)
                    tms.append(tm)
                add_dep_helper(tms[1].ins, tms[0].ins, sync=False,
                               reason="psum bank-clear order")
                atT = php.tile([P, D], bf16, tag="p0at")
                nc.scalar.activation(out=atT[:], in_=tp0[:], func=ACT.Copy)
                aw = aggp.tile([P, 4 * P], f32, tag="agg", space="PSUM")
                for j in range(2):
                    nc.tensor.matmul(out=aw[:, 0:D], lhsT=atT[:, j * P:(j + 1) * P],
                                     rhs=w1s[:, j, :], start=(j == 0), stop=(j == 1))
                abf = php.tile([P, D + 2], bf16, tag="p0w")
                nc.scalar.activation(out=abf[:, 0:D], in_=aw[:, 0:D], func=ACT.Copy)
                # riders: att_s hi/lo
                shi = php.tile([P, 1], bf16, tag="p0h")
                nc.vector.tensor_copy(out=shi[:], in_=ascol[:, ch:ch + 1])
                shif = php.tile([P, 1], f32, tag="p0hf")
                nc.vector.tensor_copy(out=shif[:], in_=shi[:])
                slo = php.tile([P, 1], f32, tag="p0l")
                nc.vector.tensor_tensor(out=slo[:], in0=ascol[:, ch:ch + 1],
                                        in1=shif[:], op=AluOpType.subtract)
                nc.vector.tensor_copy(out=abf[:, D:D + 1], in_=shi[:])
                nc.vector.tensor_copy(out=abf[:, D + 1:D + 2], in_=slo[:])
                nc.scalar.dma_start(
                    out=ctab_d[ch * P:(ch + 1) * P, 0:D + 2], in_=abf[:])

            for c in range(1, VCLS):
                nc.sync.dma_start(
                    out=ctab_d[c * CTROWS:c * CTROWS + NATT_PAD, 0:D + 2],
                    in_=ctab_d[0:NATT_PAD, 0:D + 2])

            # ---- ent scores are computed in-loop (prep_ent_*) ----
            entsc = cp.tile([P, NW], f32, tag="entsc")
            enthl = cp.tile([P, NW, 2], bf16, tag="enthl")
            ehif = cp.tile([P, NW], f32, tag="ehif")
            elo = cp.tile([P, NW], f32, tag="elo")
            ent_tiles = {}

            def prep_ent_dma(si):
                for w in sbs[si]:
                    et = entp.tile([P, D], bf16, tag="entw")
                    nc.sync.dma_start(out=et[:], in_=ent_d[w * P:(w + 1) * P, :])
                    ent_tiles[w] = et

            def prep_ent_compute(si):
                w0, w1 = sbs[si][0], sbs[si][-1] + 1
                for w in sbs[si]:
                    scr2 = php.tile([P, D], f32, tag="p0s2")
                    nc.vector.tensor_tensor(out=scr2[:], in0=ent_tiles[w][:],
                                            in1=a1r[:], op=AluOpType.mult)
                    nc.vector.reduce_sum(out=entsc[:, w:w + 1], in_=scr2[:],
                                         axis=mybir.AxisListType.X)
                nc.vector.tensor_copy(out=enthl[:, w0:w1, 0], in_=entsc[:, w0:w1])
                nc.vector.tensor_copy(out=ehif[:, w0:w1], in_=enthl[:, w0:w1, 0])
                nc.vector.tensor_tensor(out=elo[:, w0:w1], in0=entsc[:, w0:w1],
                                        in1=ehif[:, w0:w1], op=AluOpType.subtract)
                nc.vector.tensor_copy(out=enthl[:, w0:w1, 1], in_=elo[:, w0:w1])

            # ---- helpers for main loop state ----
            slot_of_call = {}
            slot_sb_issued = set([0])
            cx_of_sb = {}
            s0_of_sb = {}
            s0t_of_sb = {}
            ssb_of_sb = {}
            xr_of_sb = {}
            svt_of_sb = {}
            qn = [0]

            def issue_gathers(si):
                for ci in calls_of_sb[si]:
                    call = calls[ci]
                    nt = call["nt"]
                    cc0 = call["col0"] - sb_c0[si]
                    c = call["cls"]
                    slot = gsl.tile([P, 2 * nt, CEXT], bf16, tag="gsl",
                                    padded_shape=[P, 2 * CHUNK_T, CEXT])
                    nc.gpsimd.dma_gather(
                        out_ap=slot[:], in_ap=ctab_d[c * CTROWS:(c + 1) * CTROWS, :],
                        idxs_ap=cx_of_sb[si][:, cc0:cc0 + 16 * nt],
                        num_idxs=2 * P * nt, num_idxs_reg=nreg[2 * P * nt],
                        elem_size=CEXT, single_packet=False,
                        queue_num=qn[0] % 4)
                    qn[0] += 1
                    slot_of_call[ci] = slot

            def issue_streams(si):
                t0, t1 = sb_rng[si]
                n = (t1 - t0) * P
                s0s = s0p.tile([P, TSB * P], fp8, tag="s0s")
                nc.sync.dma_start(out=s0s[:, 0:n], in_=s0_d[:, t0 * P:t1 * P])
                s0ts = s0tp.tile([P, TSB * P], bf16, tag="s0ts")
                nc.sync.dma_start(out=s0ts[:, 0:n], in_=s0t_d[:, t0 * P:t1 * P])
                cxs = cxp.tile([P, MAXCW], mybir.dt.int16, tag="cxs")
                nc.sync.dma_start(out=cxs[:, 0:sb_cw[si]],
                                  in_=cidx_d[:, sb_c0[si]:sb_c0[si] + sb_cw[si]])
                cx_of_sb[si] = cxs
                s0_of_sb[si] = s0s
                s0t_of_sb[si] = s0ts

            def issue_xe(si):
                """Per-tile PE matvec: xr[:, ti, 0:2] = S0T_t^T @ enthl[:, w, :].
                Also reserves rowsum columns [:, TSB + wi, 0]."""
                t0, t1 = sb_rng[si]
                xr = xrp.tile([P, 2 * TSB + SBW], f32, tag="xr", space="PSUM")
                xr_of_sb[si] = xr
                s0ts = s0t_of_sb[si]
                first = None
                for t in range(t0, t1):
                    ti = t - t0
                    w = tiles[t][0]
                    wloc = w - sbs[si][0]
                    mm = nc.tensor.matmul(
                        out=xr[:, 2 * ti:2 * ti + 2],
                        lhsT=s0ts[:, ti * P:(ti + 1) * P],
                        rhs=enthl[:, w, :],
                        start=(first is None), stop=(t == t1 - 1),
                        skip_group_check=True)
                    if first is None:
                        first = mm
                    else:
                        add_dep_helper(mm.ins, first.ins, sync=False,
                                       reason="xr bank-clear order")
                return first

            xr_first_of_sb = {}

            def issue_score(si):
                """Batched score pipeline for SB si -> svt (bf16 [P, T])."""
                t0, t1 = sb_rng[si]
                T = t1 - t0
                xr = xr_of_sb[si]
                xa = scb.tile([P, TSB], f32, tag="xa")
                for ci in calls_of_sb[si]:
                    call = calls[ci]
                    slot = slot_of_call[ci]
                    r0 = call["t0"] - t0
                    nt = call["nt"]
                    sl4 = slot[:].rearrange("p (t two) e -> p t two e", two=2)
                    nc.vector.tensor_tensor(
                        out=xa[:, r0:r0 + nt],
                        in0=sl4[:, :, 0, D], in1=sl4[:, :, 0, D + 1],
                        op=AluOpType.add)
                xs1 = scb.tile([P, TSB], f32, tag="xs1")
                xrv = xr[:, 0:2 * T].rearrange("p (t two) -> p t two", two=2)
                nc.vector.tensor_tensor(out=xs1[:, 0:T], in0=xa[:, 0:T],
                                        in1=xrv[:, :, 0], op=AluOpType.add)
                xs2 = scb.tile([P, TSB], f32, tag="xs2")
                nc.vector.tensor_tensor(out=xs2[:, 0:T], in0=xs1[:, 0:T],
                                        in1=xrv[:, :, 1], op=AluOpType.add)
                x2 = scb.tile([P, TSB], f32, tag="x2")
                nc.vector.tensor_scalar(out=x2[:, 0:T], in0=xs2[:, 0:T],
                                        scalar1=0.2, scalar2=None,
                                        op0=AluOpType.mult)
                lr = scb.tile([P, TSB], f32, tag="lr")
                nc.vector.tensor_tensor(out=lr[:, 0:T], in0=xs2[:, 0:T],
                                        in1=x2[:, 0:T], op=AluOpType.max)
                svt = scb.tile([P, TSB], f32, tag="svt")
                nc.scalar.activation(out=svt[:, 0:T], in_=lr[:, 0:T],
                                     func=ACT.Exp)
                svt_of_sb[si] = svt

            def issue_sscale(si):
                t0, t1 = sb_rng[si]
                svt = svt_of_sb[si]
                s0s = s0_of_sb[si]
                ssb = ssp.tile([P, TSB * P], bf16, tag="ssb")
                for t in range(t0, t1):
                    ti = t - t0
                    nc.vector.tensor_scalar(
                        out=ssb[:, ti * P:(ti + 1) * P],
                        in0=s0s[:, ti * P:(ti + 1) * P],
                        scalar1=svt[:, ti:ti + 1], scalar2=None,
                        op0=AluOpType.mult)
                ssb_of_sb[si] = ssb

            def finalize_window(si, w, wloc, agg, agg_first, ent_tile, rr_stop):
                xr = xr_of_sb[si]
                # 1/rowsum
                rr = fip.tile([P, 1], f32, tag="rr")
                nc.vector.tensor_scalar(out=rr[:], in0=xr[:, 2 * TSB + wloc:2 * TSB + wloc + 1],
                                        scalar1=1e-30, scalar2=None,
                                        op0=AluOpType.max)
                nc.vector.reciprocal(out=rr[:], in_=rr[:])
                # val half -> transpose -> @W2 accumulated into att half
                aggs2 = fip.tile([P, D], bf16, tag="aggs2")
                nc.scalar.activation(out=aggs2[:], in_=agg[:, D:2 * D], func=ACT.Copy)
                tp = tpp.tile([P, D], f32, tag="tp", space="PSUM")
                tms = []
                for j in range(2):
                    tm = nc.tensor.matmul(out=tp[:, j * P:(j + 1) * P],
                                          lhsT=aggs2[:, j * P:(j + 1) * P],
                                          rhs=identb[:], start=(j == 0),
                                          stop=(j == 1), skip_group_check=True)
                    tms.append(tm)
                add_dep_helper(tms[1].ins, tms[0].ins, sync=False,
                               reason="psum bank-clear order")
                tps = fip.tile([P, D], bf16, tag="tps")
                nc.scalar.activation(out=tps[:], in_=tp[:], func=ACT.Copy)
                for j in range(2):
                    mm = nc.tensor.matmul(out=agg[:, 0:D],
                                          lhsT=tps[:, j * P:(j + 1) * P],
                                          rhs=w2s[:, j, :], start=False,
                                          stop=(j == 1), skip_group_check=True)
                    add_dep_helper(mm.ins, agg_first.ins, sync=False,
                                   reason="agg bank-clear order")
                # fin = agg*rr + ent ; out = elu(fin)
                fin1 = fip.tile([P, D], f32, tag="fin1")
                nc.scalar.activation(out=fin1[:], in_=agg[:, 0:D], func=ACT.Copy,
                                     scale=rr[:])
                fin2 = fip.tile([P, D], f32, tag="fin2")
                nc.vector.tensor_tensor(out=fin2[:], in0=fin1[:], in1=ent_tile[:],
                                        op=AluOpType.add)
                rng = fip.tile([P, D], f32, tag="rng")
                nc.scalar.activation(out=rng[:], in_=fin2[:], func=ACT.Relu,
                                     scale=-1.0)
                ex = fip.tile([P, D], f32, tag="ex")
                nc.scalar.activation(out=ex[:], in_=rng[:], func=ACT.Exp,
                                     scale=-1.0)
                rl = fip.tile([P, D], f32, tag="rl")
                nc.scalar.activation(out=rl[:], in_=fin2[:], func=ACT.Relu)
                fo = fip.tile([P, D], f32, tag="fo")
                nc.vector.tensor_tensor(out=fo[:], in0=rl[:], in1=ex[:],
                                        op=AluOpType.add)
                fo2 = fip.tile([P, D], f32, tag="fo2")
                nc.scalar.activation(out=fo2[:], in_=fo[:], func=ACT.Copy,
                                     bias=-1.0)
                nc.scalar.dma_start(out=out_d[w * P:(w + 1) * P, :], in_=fo2[:])

            # ---- prologue ----
            max_sbs = int(os.environ.get("K_MAX_SBS", "9999"))
            pro = int(os.environ.get("K_PROLOGUE", "7"))
            for s in range(min(3, nsb)):
                prep_ent_dma(s)
            for s in range(min(3, nsb)):
                prep_ent_compute(s)
            for s in range(min(2, nsb)):
                issue_streams(s)
            issue_gathers(0)
            xr_first_of_sb[0] = issue_xe(0)

            # ---- main loop ----
            for si, sb in enumerate(sbs):
                if si >= max_sbs:
                    break
                t0, t1 = sb_rng[si]
                if si + 1 < nsb and si + 1 not in s0_of_sb:
                    issue_streams(si + 1)
                if si + 3 < nsb:
                    prep_ent_dma(si + 3)
                issue_score(si)
                issue_sscale(si)
                if si + 3 < nsb:
                    prep_ent_compute(si + 3)
                ssb = ssb_of_sb[si]
                xr = xr_of_sb[si]
                pending = []
                agg_of_w = {}
                aggfirst_of_w = {}
                for wi, w in enumerate(sb):
                    agg = aggp.tile([P, 4 * P], f32, tag="agg", space="PSUM")
                    agg_of_w[w] = agg
                    wtiles = [t for t in range(t0, t1) if tiles[t][0] == w]
                    firstmm = None
                    for t in wtiles:
                        ti = t - t0
                        ci, b = call_of_tile[t]
                        slot = slot_of_call[ci]
                        mm = nc.tensor.matmul(out=agg[:, 0:4 * P],
                                              lhsT=ssb[:, ti * P:(ti + 1) * P],
                                              rhs=slot[:, 2 * b:2 * b + 2, 0:D],
                                              start=(firstmm is None),
                                              stop=False, skip_group_check=True)
                        if firstmm is None:
                            firstmm = mm
                        else:
                            add_dep_helper(mm.ins, firstmm.ins, sync=False,
                                           reason="agg bank-clear order")
                        is_last_rs = (wi == len(sb) - 1 and t == wtiles[-1])
                        rs = nc.tensor.matmul(out=xr[:, 2 * TSB + wi:2 * TSB + wi + 1],
                                              lhsT=ssb[:, ti * P:(ti + 1) * P],
                                              rhs=ones_mm[:], start=False,
                                              stop=is_last_rs,
                                              skip_group_check=True)
                        add_dep_helper(rs.ins, xr_first_of_sb[si].ins, sync=False,
                                       reason="xr bank-clear order")
                    aggfirst_of_w[w] = firstmm
                    if si + 1 < nsb and wi == 0 and si + 1 not in slot_sb_issued:
                        issue_gathers(si + 1)
                        slot_sb_issued.add(si + 1)
                    if si + 1 < nsb and wi == 0 and si + 1 not in xr_first_of_sb:
                        xr_first_of_sb[si + 1] = issue_xe(si + 1)
                    if pending:
                        pw, pwloc = pending.pop()
                        finalize_window(si, pw, pwloc, agg_of_w[pw],
                                        aggfirst_of_w[pw], ent_tiles[pw], None)
                    pending.append((w, wi))
                if pending:
                    pw, pwloc = pending.pop()
                    finalize_window(si, pw, pwloc, agg_of_w[pw],
                                    aggfirst_of_w[pw], ent_tiles[pw], None)

    lower_extended_insts(nc)
    reassign_swdge_queues(nc)
    legalize_waits(nc)
    return nc
def _per_core_inputs(sched, shards, att_feats, val_feats, ent_feats, a_w, a_b, W):
    import ml_dtypes
    NT = sched["n_tiles"]
    attf = np.zeros((NATT_PAD, D), dtype=np.float32)
    attf[:N_ATT] = np.asarray(att_feats, dtype=np.float32)
    ctab = np.zeros((VCLS * CTROWS, CEXT), dtype=ml_dtypes.bfloat16)
    valbf = np.asarray(val_feats, dtype=np.float32).astype(ml_dtypes.bfloat16)
    for c in range(VCLS):
        nrow = min(VSH, N_VAL - c * VSH)
        ctab[c * CTROWS + NATT_PAD:c * CTROWS + NATT_PAD + nrow, 0:D] = \
            valbf[c * VSH:c * VSH + nrow]
    entp_full = np.zeros((N_CORES * NODES_PER_CORE, D), dtype=ml_dtypes.bfloat16)
    entp_full[:N_ENT] = np.asarray(ent_feats,
                                   dtype=np.float32).astype(ml_dtypes.bfloat16)
    a_w = np.asarray(a_w, dtype=np.float32)
    a1r = np.tile(a_w[0, :D][None, :], (P, 1)).astype(np.float32)
    a2r = np.tile(a_w[0, D:][None, :], (P, 1)).astype(np.float32)
    br = np.full((P, 1), float(np.asarray(a_b).reshape(-1)[0]), dtype=np.float32)
    Wf = np.asarray(W, dtype=np.float32)
    w1 = Wf[:D].astype(ml_dtypes.bfloat16)
    w2 = Wf[D:].astype(ml_dtypes.bfloat16)

    in_maps = []
    for k in range(N_CORES):
        w_arr, hrel_arr, cls_arr, vloc_arr, att_arr = shards[k]
        hrelf = np.full((NT, P), -1, dtype=np.int64)   # -1 = pad
        vli = np.zeros((NT, P), dtype=np.int64)
        ati = np.zeros((NT, P), dtype=np.int64)
        order = np.lexsort((cls_arr, w_arr))
        wc_sorted = list(zip(w_arr[order], cls_arr[order]))
        i = 0
        nE = len(order)
        while i < nE:
            w0, c0 = wc_sorted[i]
            j = i
            while j < nE and wc_sorted[j] == (w0, c0):
                j += 1
            idxs = order[i:j]
            tlist = sched["tile_of"][(w0, c0)]
            assert len(idxs) <= len(tlist) * P, (k, w0, c0, len(idxs))
            for q, e in enumerate(idxs):
                t = tlist[q // P]
                p = q % P
                hrelf[t, p] = hrel_arr[e]
                vli[t, p] = vloc_arr[e]
                ati[t, p] = att_arr[e]
            i = j
        cidx = np.zeros((P, sched["icols"]), dtype=np.int16)
        for call in sched["calls"]:
            t0, nt, c0 = call["t0"], call["nt"], call["col0"]
            flat = np.stack([ati[t0:t0 + nt],
                             NATT_PAD + vli[t0:t0 + nt]], axis=1).reshape(-1)
            cidx[:, c0:c0 + 16 * nt] = _pack16(flat)
        # one-hot streams
        s0 = np.zeros((P, NT * P), dtype=ml_dtypes.float8_e4m3)
        s0t = np.zeros((P, NT * P), dtype=ml_dtypes.bfloat16)
        tt, pp = np.nonzero(hrelf >= 0)
        hh = hrelf[tt, pp]
        s0[pp, tt * P + hh] = 1.0
        s0t[hh, tt * P + pp] = 1.0
        in_maps.append(dict(
            ent=entp_full[k * NODES_PER_CORE:(k + 1) * NODES_PER_CORE],
            attf=attf, ctab=ctab, a1r=a1r, a2r=a2r, br=br,
            w1=w1, w2=w2, cidx=cidx, s0=s0, s0t=s0t,
        ))
    return in_maps


def kernel(attribute_triples, att_feats, val_feats, ent_feats, a_w, a_b, W):
    shards = _host_shard(attribute_triples, ent_feats)
    counts = []
    for k in range(N_CORES):
        w_arr, hrel_arr, cls_arr, vloc_arr, att_arr = shards[k]
        cnt = np.zeros((NW, VCLS), dtype=np.int64)
        np.add.at(cnt, (w_arr, cls_arr), 1)
        counts.append(cnt)
    sched = _schedule(counts)

    nc = build_program(sched)
    in_maps = _per_core_inputs(sched, shards, att_feats, val_feats, ent_feats,
                               a_w, a_b, W)
    trace = os.environ.get("KERNEL_TRACE", "0") == "1"
    res = _bu.run_bass_kernel_spmd(nc, in_maps, list(range(N_CORES)), trace=trace)
    if trace and res.exec_time_ns:
        print(f"HW exec time: {res.exec_time_ns} ns")
    out = np.concatenate([res.results[k]["out"] for k in range(N_CORES)], axis=0)
    return np.ascontiguousarray(out[:N_ENT]).astype(np.float32)
